# revision 1
# baseline (speedup 1.0000x reference)
"""Trainium2 Bass kernel for the ATripletMarginLossOHNMDM loss.

Per row i of an (B, B) input:
  sim_p      = input[i, i]
  masked     = where(target[i]==0, input[i], -1e9)
  sim_n[0:3] = top-3 values of masked          (hard negatives)
  d          = clip(|sim_p - sim_n|, 0.1, 0.3)
  loss       = relu(sim_n - sim_p + d)
  s          = where(loss>0, sim_n, -50)
  w          = softmax(s / 0.1)      (with max-subtraction, as jax.nn.softmax)
  out        = mean over (B, 3) of loss * w

Sharded by rows across 8 NeuronCores (1024 rows each). Per core, per
128-row tile:
  - DMA the input rows (f32) and the target rows. The target holds only
    0/1, so the host passes `target.view(int8)[:, ::4]` — a strided byte
    view of the original int32 buffer (pure data movement, no
    arithmetic) — which cuts target DMA traffic 4x.
  - the mask  m = (target * -1e9) + input  is applied in place: a DVE
    scalar_tensor_tensor covers most columns while a GPSIMD
    tensor_tensor pair covers a 2048-column slice, phase-locked (via an
    explicit dep) into the window where the DVE runs Max8 and its
    shared SBUF port pair is free
  - one DVE Max8 instruction returns the top-8 per row -> top-3,
    collected into a [128, n_tiles, 8] buffer
Tile 0 is processed in four column chunks so DVE work starts as soon as
the first chunk lands (chunking more tiles, or coalescing input+target
into one transfer, measured worse: DMA FIFO first-byte latency and
completion tails dominate such reshuffles). Tile 1 runs the mask
DVE-only to seed the GPSIMD phase chain. A single
vectorized epilogue then computes the margin/softmax math for all tiles
at once on [128, n_tiles, 3], and the per-(partition, tile) partial
sums are DMA'd out as [128, n_tiles]. The final mean over the
8 * 128 * n_tiles partials is computed on host.

Measured on 8 axon-tunneled trn2 cores: ~155 us HW exec on
nominal-clock devices (178 us on slower-clock pool allocations; all
engine ops scale ~1.2x there), relative error 0.0 vs the jax
reference. Roofline context: 40 MiB/core of mandatory DMA traffic
(~100 us at line rate) and ~131 us of DVE work (mask + Max8), with the
steady-state loop running gap-free on the DVE at 15.15 us/tile.
"""

import numpy as np

import concourse.bacc as bacc
import concourse.mybir as mybir
import concourse.tile as tile
from concourse.bass_utils import run_bass_kernel_spmd

_B = 8192          # full problem size (rows == cols)
_NCORES = 8
_P = 128           # SBUF partitions
_K = 3
_BIG_NEG = -1.0e9  # mask fill; far below any real similarity
_NEG_FILL = -50.0  # reference's softmax mask fill (must match exactly)
_INV_TAU = 10.0    # 1 / 0.1
# Columns of the mask pass offloaded to GPSIMD (as two tensor_tensor ops —
# the fused scalar_tensor_tensor opcode is not legal on Pool). Pool ops take
# the SBUF port pair shared with the DVE, so the chunk is sized to run inside
# the DVE Max8 window (Max8 only uses the DVE-dedicated port).
_GP_COLS = 2048


def _build_nc(rows_per_core: int, ncols: int) -> bacc.Bacc:
    n_tiles = rows_per_core // _P
    f32 = mybir.dt.float32
    i32 = mybir.dt.int32

    nc = bacc.Bacc()
    inp = nc.dram_tensor("inp", [rows_per_core, ncols], f32,
                         kind="ExternalInput")
    tgt = nc.dram_tensor("tgt", [rows_per_core, ncols], mybir.dt.int8,
                         kind="ExternalInput")
    # diag[p, t] = input diagonal element of local row t*128 + p
    diag = nc.dram_tensor("diag", [_P, n_tiles], f32, kind="ExternalInput")
    out = nc.dram_tensor("out", [_P, n_tiles], f32, kind="ExternalOutput")

    with tile.TileContext(nc) as tc:
        with (
            tc.tile_pool(name="singles", bufs=1) as singles,
            tc.tile_pool(name="io_in", bufs=3) as io_in,
            tc.tile_pool(name="io_tg", bufs=3) as io_tg,
            tc.tile_pool(name="gp", bufs=2) as gp_pool,
            tc.tile_pool(name="small", bufs=1) as small,
        ):
            negbig = singles.tile([_P, 1], f32)
            nc.vector.memset(negbig, _BIG_NEG)
            # top-8 per (row, tile), filled by the main loop
            vfin = singles.tile([_P, n_tiles, 8], f32)

            # Only tile 0 is processed in column chunks (DVE-only) so
            # compute starts as soon as the first chunk lands. Chunking
            # more tiles backfires: each extra DMA costs ~0.6us of HWDGE
            # first-byte latency on the FIFO, delaying later tiles' data.
            chunk0 = ncols >= 4096 and ncols % 4 == 0
            vcol0 = None
            if chunk0:
                vcol0 = singles.tile([_P, 4, 8], f32)
            prev_stt = None

            for t in range(n_tiles):
                rows = slice(t * _P, (t + 1) * _P)
                in_t = io_in.tile([_P, ncols], f32)
                tg_t = io_tg.tile([_P, ncols], mybir.dt.int8)
                if t == 0 and chunk0:
                    w = ncols // 4
                    for c in range(4):
                        cs = slice(c * w, (c + 1) * w)
                        nc.sync.dma_start(out=in_t[:, cs], in_=inp[rows, cs])
                        nc.sync.dma_start(out=tg_t[:, cs], in_=tgt[rows, cs])
                    for c in range(4):
                        cs = slice(c * w, (c + 1) * w)
                        nc.vector.scalar_tensor_tensor(
                            out=in_t[:, cs], in0=tg_t[:, cs], scalar=_BIG_NEG,
                            in1=in_t[:, cs],
                            op0=mybir.AluOpType.mult, op1=mybir.AluOpType.add)
                        nc.vector.max(out=vcol0[:, c, :], in_=in_t[:, cs])
                    nc.vector.max(out=vfin[:, t, :], in_=vcol0[:, :, :])
                    continue
                nc.sync.dma_start(out=in_t, in_=inp[rows, :])
                nc.sync.dma_start(out=tg_t, in_=tgt[rows, :])
                # masked = (target * -1e9) + input, in place; the column
                # range is split between GPSIMD and DVE (see _GP_COLS).
                # Tile 1 is DVE-only: its Max8 hosts the first phase window.
                gp_cols = min(_GP_COLS, ncols // 4) if t >= 2 else 0
                sp = ncols - gp_cols
                if gp_cols:
                    ug = gp_pool.tile([_P, gp_cols], f32, tag="ug")
                    gp_mul = nc.gpsimd.tensor_tensor(
                        out=ug, in0=tg_t[:, sp:],
                        in1=negbig.to_broadcast([_P, gp_cols]),
                        op=mybir.AluOpType.mult)
                    if prev_stt is not None:
                        # Pool ops take the SBUF port pair shared with the
                        # DVE; gate them behind the previous tile's DVE mask
                        # op so they run inside the Max8 window (Max8 only
                        # uses the DVE-dedicated port) instead of blocking
                        # the next DVE mask op mid-instruction.
                        tile.add_dep_helper(
                            gp_mul.ins, prev_stt.ins,
                            reason="phase GPSIMD into the Max8 port window")
                    nc.gpsimd.tensor_tensor(
                        out=in_t[:, sp:], in0=in_t[:, sp:], in1=ug,
                        op=mybir.AluOpType.add)
                prev_stt = nc.vector.scalar_tensor_tensor(
                    out=in_t[:, :sp], in0=tg_t[:, :sp], scalar=_BIG_NEG,
                    in1=in_t[:, :sp],
                    op0=mybir.AluOpType.mult, op1=mybir.AluOpType.add)
                nc.vector.max(out=vfin[:, t, :], in_=in_t)

            # ---- vectorized epilogue over all tiles: [128, n_tiles, 3] ----
            diag_raw = singles.tile([_P, n_tiles], f32)
            nc.sync.dma_start(out=diag_raw, in_=diag[:, :])
            diag_sb = singles.tile([_P, n_tiles], f32)
            nc.vector.tensor_copy(out=diag_sb, in_=diag_raw)
            sh = [_P, n_tiles, _K]
            v = small.tile(sh, f32)                    # top-3, descending
            nc.vector.tensor_copy(out=v, in_=vfin[:, :, 0:_K])
            p_b = diag_sb.unsqueeze(-1).to_broadcast(sh)

            x = small.tile(sh, f32)                    # x = sim_n - sim_p
            nc.vector.tensor_tensor(out=x, in0=v, in1=p_b,
                                    op=mybir.AluOpType.subtract)
            # a = clip(|x|, 0.1, 0.3)   (|x| as max(x, -x), bitwise exact)
            negx = small.tile(sh, f32)
            nc.vector.tensor_scalar(out=negx, in0=x, scalar1=-1.0,
                                    scalar2=None, op0=mybir.AluOpType.mult)
            a = small.tile(sh, f32)
            nc.vector.tensor_tensor(out=a, in0=x, in1=negx,
                                    op=mybir.AluOpType.max)
            nc.vector.tensor_scalar(out=a, in0=a, scalar1=0.1, scalar2=0.3,
                                    op0=mybir.AluOpType.max,
                                    op1=mybir.AluOpType.min)
            # loss = relu(x + a); active = (x + a) > 0
            xa = small.tile(sh, f32)
            nc.vector.tensor_tensor(out=xa, in0=x, in1=a,
                                    op=mybir.AluOpType.add)
            l = small.tile(sh, f32)
            nc.vector.tensor_scalar(out=l, in0=xa, scalar1=0.0, scalar2=None,
                                    op0=mybir.AluOpType.max)
            act = small.tile(sh, i32)
            nc.vector.tensor_scalar(out=act, in0=xa, scalar1=0.0, scalar2=None,
                                    op0=mybir.AluOpType.is_gt)
            # s = where(active, v, -50)
            s = small.tile(sh, f32)
            nc.vector.memset(s, _NEG_FILL)
            nc.vector.copy_predicated(out=s, mask=act, data=v)
            # softmax(s / tau) over K, with max-subtraction (matches jax)
            smax = small.tile([_P, n_tiles], f32)
            nc.vector.reduce_max(out=smax, in_=s, axis=mybir.AxisListType.X)
            s2 = small.tile(sh, f32)
            nc.vector.tensor_tensor(out=s2, in0=s,
                                    in1=smax.unsqueeze(-1).to_broadcast(sh),
                                    op=mybir.AluOpType.subtract)
            e = small.tile(sh, f32)
            nc.scalar.activation(out=e, in_=s2,
                                 func=mybir.ActivationFunctionType.Exp,
                                 scale=_INV_TAU)
            z = small.tile([_P, n_tiles], f32)
            nc.vector.reduce_sum(out=z, in_=e, axis=mybir.AxisListType.X)
            r = small.tile([_P, n_tiles], f32)
            nc.vector.reciprocal(out=r, in_=z)
            w = small.tile(sh, f32)
            nc.vector.tensor_tensor(out=w, in0=e,
                                    in1=r.unsqueeze(-1).to_broadcast(sh),
                                    op=mybir.AluOpType.mult)
            lw = small.tile(sh, f32)
            nc.vector.tensor_tensor(out=lw, in0=l, in1=w,
                                    op=mybir.AluOpType.mult)
            out_sb = small.tile([_P, n_tiles], f32)
            nc.vector.reduce_sum(out=out_sb, in_=lw, axis=mybir.AxisListType.X)
            nc.sync.dma_start(out=out[:, :], in_=out_sb)
    nc.compile()
    return nc


def _prepare_in_maps(inp: np.ndarray, tgt: np.ndarray, ncores: int):
    b, ncols = inp.shape
    rows = b // ncores
    n_tiles = rows // _P
    d = np.ascontiguousarray(np.diagonal(inp)).astype(np.float32, copy=False)
    # 0/1 int32 little-endian: byte 0 of each element carries the value
    tgt_v = np.ascontiguousarray(tgt.view(np.int8)[:, ::4])
    in_maps = []
    for c in range(ncores):
        sl = slice(c * rows, (c + 1) * rows)
        diag_c = np.ascontiguousarray(d[sl].reshape(n_tiles, _P).T)
        in_maps.append({
            "inp": np.ascontiguousarray(inp[sl]),
            "tgt": np.ascontiguousarray(tgt_v[sl]),
            "diag": diag_c,
        })
    return in_maps


_NC_CACHE = {}


def kernel(input, target):
    inp = np.asarray(input, dtype=np.float32)
    tgt = np.asarray(target, dtype=np.int32)
    b, ncols = inp.shape

    key = (b, ncols)
    nc = _NC_CACHE.get(key)
    if nc is None:
        nc = _NC_CACHE[key] = _build_nc(b // _NCORES, ncols)
    in_maps = _prepare_in_maps(inp, tgt, _NCORES)
    res = run_bass_kernel_spmd(nc, in_maps, list(range(_NCORES)))
    total = 0.0
    for r in res.results:
        total += r["out"].astype(np.float64).sum()
    return np.asarray(total / (b * _K), dtype=np.float32)


if __name__ == "__main__":
    rng = np.random.default_rng(0)
    b = _B
    x = rng.standard_normal((b, b), dtype=np.float32)
    t = rng.integers(0, 2, size=(b, b)).astype(np.int32)
    np.fill_diagonal(t, 1)
    print(kernel(x, t))



# revision 2
# speedup vs baseline: 1.4037x; 1.4037x over previous
"""Trainium2 Bass kernel for the ATripletMarginLossOHNMDM loss.

Per row i of an (B, B) input:
  sim_p      = input[i, i]
  masked     = where(target[i]==0, input[i], -1e9)
  sim_n[0:3] = top-3 values of masked          (hard negatives)
  d          = clip(|sim_p - sim_n|, 0.1, 0.3)
  loss       = relu(sim_n - sim_p + d)
  s          = where(loss>0, sim_n, -50)
  w          = softmax(s / 0.1)      (with max-subtraction, as jax.nn.softmax)
  out        = mean over (B, 3) of loss * w

Sharded by rows across 8 NeuronCores (1024 rows each). The rel-err
budget (2e-2) is far looser than bf16 rounding (~0.4% per value), so the
whole selection pipeline runs in bf16, which unlocks the DVE 2x packed
mode (fp32 runs at 1 elem/cycle/lane; Max8 never gets a fast mode at
all, so feeding it all 8192 columns — what the first version of this
kernel did — costs 8.5us/tile on its own).

Per core, per 128-row tile [128, 8192]:
  - DMA the input rows as bf16 (host-cast; 2 bytes/elem) and the target
    rows as int8 (host passes target.view(int8)[:, ::4], a strided byte
    view of the 0/1 int32 buffer — pure data movement).
  - the otherwise-idle Scalar engine decodes the target:
    tgtbf = Copy(-1e9 * tgt) -> bf16   (7.0us/tile, under the DVE time)
  - DVE applies the mask as one 2x tensor_tensor add (x += tgtbf,
    4096 cyc), then folds the row max hierarchy in place
    8192 -> 4096 -> 2048 -> 1024 -> 512 (four 2x tensor_tensor max ops,
    3840 cyc total) and runs Max8 on the surviving 512 columns
    (570 cyc). Folding by halves keeps every operand unit-stride so all
    folds stay in the 2x packed mode.
  - Fold slot j aggregates columns {j, j+512, ..., j+7680}; top-3 values
    of the slot maxima miss a true top-3 element only when two of them
    collide in one slot (P ~ 0.5% of rows, and the miss substitutes the
    4th-largest negative — sub-1e-4 effect on the final mean).
Tile 0 is processed in four 2048-column chunks (mask + fold to 512 +
Max8 per chunk, then an 8-from-32 Max8) so DVE work starts as soon as
the first chunk lands. A single vectorized epilogue then computes the
margin/softmax math for all tiles at once on [128, n_tiles, 3] in f32,
and the per-(partition, tile) partial sums are DMA'd out as
[128, n_tiles]. The final mean over the 8 * 128 * n_tiles partials is
computed on host. sim_p comes from a separately-DMA'd exact f32
diagonal.

Budget (per core): DMA 24 MiB (~70us at the ~358 GB/s HBM-per-core
limit), DVE ~9.4us/tile (~76us), Act ~7us/tile (~56us).
"""

import numpy as np
import ml_dtypes

import concourse.bacc as bacc
import concourse.mybir as mybir
import concourse.tile as tile
from concourse.bass_utils import run_bass_kernel_spmd

_B = 8192          # full problem size (rows == cols)
_NCORES = 8
_P = 128           # SBUF partitions
_K = 3
_BIG_NEG = -1.0e9  # mask fill; far below any real similarity
_NEG_FILL = -50.0  # reference's softmax mask fill (must match exactly)
_INV_TAU = 10.0    # 1 / 0.1
_FOLD_W = 512      # fold the row down to this width before Max8


def _mask_fold_max(nc, x, tgtbf, width, out8):
    """masked = x + tgtbf (in place), fold max by halves to _FOLD_W,
    Max8 into out8. All bf16, unit-stride => DVE 2x packed mode."""
    nc.vector.tensor_tensor(out=x[:, :width], in0=x[:, :width],
                            in1=tgtbf[:, :width], op=mybir.AluOpType.add)
    w = width
    while w > _FOLD_W:
        h = w // 2
        nc.vector.tensor_tensor(out=x[:, :h], in0=x[:, :h], in1=x[:, h:w],
                                op=mybir.AluOpType.max)
        w = h
    nc.vector.max(out=out8, in_=x[:, :w])


def _build_nc(rows_per_core: int, ncols: int) -> bacc.Bacc:
    n_tiles = rows_per_core // _P
    f32 = mybir.dt.float32
    bf16 = mybir.dt.bfloat16
    i32 = mybir.dt.int32

    nc = bacc.Bacc()
    inp = nc.dram_tensor("inp", [rows_per_core, ncols], bf16,
                         kind="ExternalInput")
    tgt = nc.dram_tensor("tgt", [rows_per_core, ncols], mybir.dt.int8,
                         kind="ExternalInput")
    # diag[p, t] = input diagonal element of local row t*128 + p
    diag = nc.dram_tensor("diag", [_P, n_tiles], f32, kind="ExternalInput")
    out = nc.dram_tensor("out", [_P, n_tiles], f32, kind="ExternalOutput")

    with tile.TileContext(nc) as tc:
        with (
            tc.tile_pool(name="singles", bufs=1) as singles,
            tc.tile_pool(name="io_in", bufs=3) as io_in,
            tc.tile_pool(name="io_tg", bufs=3) as io_tg,
            tc.tile_pool(name="tb", bufs=2) as tb_pool,
            tc.tile_pool(name="small", bufs=1) as small,
        ):
            # top-8 per (row, tile), filled by the main loop
            vfin = singles.tile([_P, n_tiles, 8], bf16)

            # Only tile 0 is processed in column chunks so compute starts
            # as soon as the first chunk lands. Chunking more tiles
            # backfires: each extra DMA costs ~0.6us of HWDGE first-byte
            # latency on the FIFO, delaying later tiles' data.
            chunk0 = ncols >= 4096 and ncols % 4 == 0
            vcol0 = None
            if chunk0:
                vcol0 = singles.tile([_P, 4, 8], bf16)

            for t in range(n_tiles):
                rows = slice(t * _P, (t + 1) * _P)
                in_t = io_in.tile([_P, ncols], bf16)
                tg_t = io_tg.tile([_P, ncols], mybir.dt.int8)
                tgtbf = tb_pool.tile([_P, ncols], bf16, tag="tb")
                if t == 0 and chunk0:
                    w = ncols // 4
                    for c in range(4):
                        cs = slice(c * w, (c + 1) * w)
                        nc.sync.dma_start(out=in_t[:, cs], in_=inp[rows, cs])
                        nc.sync.dma_start(out=tg_t[:, cs], in_=tgt[rows, cs])
                    for c in range(4):
                        cs = slice(c * w, (c + 1) * w)
                        nc.scalar.mul(out=tgtbf[:, cs], in_=tg_t[:, cs],
                                      mul=_BIG_NEG)
                        _mask_fold_max(nc, in_t[:, cs], tgtbf[:, cs], w,
                                       vcol0[:, c, :])
                    nc.vector.max(out=vfin[:, t, :], in_=vcol0[:, :, :])
                    continue
                nc.sync.dma_start(out=in_t, in_=inp[rows, :])
                nc.sync.dma_start(out=tg_t, in_=tgt[rows, :])
                nc.scalar.mul(out=tgtbf, in_=tg_t, mul=_BIG_NEG)
                _mask_fold_max(nc, in_t, tgtbf, ncols, vfin[:, t, :])

            # ---- vectorized epilogue over all tiles: [128, n_tiles, 3] ----
            diag_raw = singles.tile([_P, n_tiles], f32)
            nc.sync.dma_start(out=diag_raw, in_=diag[:, :])
            diag_sb = singles.tile([_P, n_tiles], f32)
            nc.vector.tensor_copy(out=diag_sb, in_=diag_raw)
            sh = [_P, n_tiles, _K]
            v = small.tile(sh, f32)                    # top-3, descending
            nc.vector.tensor_copy(out=v, in_=vfin[:, :, 0:_K])
            p_b = diag_sb.unsqueeze(-1).to_broadcast(sh)

            x = small.tile(sh, f32)                    # x = sim_n - sim_p
            nc.vector.tensor_tensor(out=x, in0=v, in1=p_b,
                                    op=mybir.AluOpType.subtract)
            # a = clip(|x|, 0.1, 0.3)   (|x| as max(x, -x), bitwise exact)
            negx = small.tile(sh, f32)
            nc.vector.tensor_scalar(out=negx, in0=x, scalar1=-1.0,
                                    scalar2=None, op0=mybir.AluOpType.mult)
            a = small.tile(sh, f32)
            nc.vector.tensor_tensor(out=a, in0=x, in1=negx,
                                    op=mybir.AluOpType.max)
            nc.vector.tensor_scalar(out=a, in0=a, scalar1=0.1, scalar2=0.3,
                                    op0=mybir.AluOpType.max,
                                    op1=mybir.AluOpType.min)
            # loss = relu(x + a); active = (x + a) > 0
            xa = small.tile(sh, f32)
            nc.vector.tensor_tensor(out=xa, in0=x, in1=a,
                                    op=mybir.AluOpType.add)
            l = small.tile(sh, f32)
            nc.vector.tensor_scalar(out=l, in0=xa, scalar1=0.0, scalar2=None,
                                    op0=mybir.AluOpType.max)
            act = small.tile(sh, i32)
            nc.vector.tensor_scalar(out=act, in0=xa, scalar1=0.0, scalar2=None,
                                    op0=mybir.AluOpType.is_gt)
            # s = where(active, v, -50)
            s = small.tile(sh, f32)
            nc.vector.memset(s, _NEG_FILL)
            nc.vector.copy_predicated(out=s, mask=act, data=v)
            # softmax(s / tau) over K, with max-subtraction (matches jax)
            smax = small.tile([_P, n_tiles], f32)
            nc.vector.reduce_max(out=smax, in_=s, axis=mybir.AxisListType.X)
            s2 = small.tile(sh, f32)
            nc.vector.tensor_tensor(out=s2, in0=s,
                                    in1=smax.unsqueeze(-1).to_broadcast(sh),
                                    op=mybir.AluOpType.subtract)
            e = small.tile(sh, f32)
            nc.scalar.activation(out=e, in_=s2,
                                 func=mybir.ActivationFunctionType.Exp,
                                 scale=_INV_TAU)
            z = small.tile([_P, n_tiles], f32)
            nc.vector.reduce_sum(out=z, in_=e, axis=mybir.AxisListType.X)
            r = small.tile([_P, n_tiles], f32)
            nc.vector.reciprocal(out=r, in_=z)
            w = small.tile(sh, f32)
            nc.vector.tensor_tensor(out=w, in0=e,
                                    in1=r.unsqueeze(-1).to_broadcast(sh),
                                    op=mybir.AluOpType.mult)
            lw = small.tile(sh, f32)
            nc.vector.tensor_tensor(out=lw, in0=l, in1=w,
                                    op=mybir.AluOpType.mult)
            out_sb = small.tile([_P, n_tiles], f32)
            nc.vector.reduce_sum(out=out_sb, in_=lw, axis=mybir.AxisListType.X)
            nc.sync.dma_start(out=out[:, :], in_=out_sb)
    nc.compile()
    return nc


def _prepare_in_maps(inp: np.ndarray, tgt: np.ndarray, ncores: int):
    b, ncols = inp.shape
    rows = b // ncores
    n_tiles = rows // _P
    d = np.ascontiguousarray(np.diagonal(inp)).astype(np.float32, copy=False)
    # 0/1 int32 little-endian: byte 0 of each element carries the value
    tgt_v = np.ascontiguousarray(tgt.view(np.int8)[:, ::4])
    in_maps = []
    for c in range(ncores):
        sl = slice(c * rows, (c + 1) * rows)
        diag_c = np.ascontiguousarray(d[sl].reshape(n_tiles, _P).T)
        in_maps.append({
            "inp": inp[sl].astype(ml_dtypes.bfloat16),
            "tgt": np.ascontiguousarray(tgt_v[sl]),
            "diag": diag_c,
        })
    return in_maps


_NC_CACHE = {}


def kernel(input, target):
    inp = np.asarray(input, dtype=np.float32)
    tgt = np.asarray(target, dtype=np.int32)
    b, ncols = inp.shape

    key = (b, ncols)
    nc = _NC_CACHE.get(key)
    if nc is None:
        nc = _NC_CACHE[key] = _build_nc(b // _NCORES, ncols)
    in_maps = _prepare_in_maps(inp, tgt, _NCORES)
    res = run_bass_kernel_spmd(nc, in_maps, list(range(_NCORES)))
    total = 0.0
    for r in res.results:
        total += r["out"].astype(np.float64).sum()
    return np.asarray(total / (b * _K), dtype=np.float32)


if __name__ == "__main__":
    rng = np.random.default_rng(0)
    b = _B
    x = rng.standard_normal((b, b), dtype=np.float32)
    t = rng.integers(0, 2, size=(b, b)).astype(np.int32)
    np.fill_diagonal(t, 1)
    print(kernel(x, t))


# revision 3
# speedup vs baseline: 1.6800x; 1.1968x over previous
"""Trainium2 Bass kernel for the ATripletMarginLossOHNMDM loss.

Per row i of an (B, B) input:
  sim_p      = input[i, i]
  masked     = where(target[i]==0, input[i], -big)
  sim_n[0:3] = top-3 values of masked          (hard negatives)
  d          = clip(|sim_p - sim_n|, 0.1, 0.3)
  loss       = relu(sim_n - sim_p + d)
  s          = where(loss>0, sim_n, -50)
  w          = softmax(s / 0.1)      (with max-subtraction, as jax.nn.softmax)
  out        = mean over (B, 3) of loss * w

Sharded by rows across 8 NeuronCores (1024 rows each). The rel-err
budget (2e-2) dwarfs fp8-e4m3 rounding of the candidate values
(measured 1.0e-3 end to end on the real inputs), which unlocks the
memory-bound optimum: ship the input as fp8 and the 0/1 target as fp8,
16 MiB/core instead of the f32+int8 40 MiB — the HBM-per-core limit
(~358 GB/s) makes DMA the roofline at ~47us.

The mask runs on the otherwise-idle TensorEngine as ONE DoubleRow fp8
matmul per 512-column chunk: lhsT = [Id; -240*Id] (fp8, [128,2,128]),
rhs = [x8; t8] ([128,2,512] slices of the host-interleaved input), so
PSUM accumulates masked = x - 240*t in f32 exactly (Id picks x through
the array; -240 is the e4m3 max, far below any N(0,1) similarity).
DoubleRow virtualizes the PE to 128x256 contraction so both terms cost
a single streaming pass.

The PSUM->SBUF eviction (1 elem/cycle/lane on any engine) is the
structural wall: 8192 f32/tile split between the Scalar engine
(activation Copy -> bf16, 3.5 of 4 PSUM quarters) and the DVE
(tensor_copy, the last 1024 columns), ~6.5us/tile each. The DVE then
folds the row max hierarchy in place in bf16 2x packed mode
(8192 -> 512 by halves, 3840 cyc) and Max8s the surviving 512 columns
(~9x less Max8 than scanning the full row, which has no fast mode).
Fold slot j aggregates columns {j, j+512, ...}; a true top-3 element is
missed only when two of them collide in one slot (~0.5% of rows, miss
substitutes the 4th-largest — sub-1e-4 effect on the final mean).

Tile 0 is processed per 2048-column quarter (DMA -> 4 matmuls -> evict)
so PE work starts as soon as the first quarter lands. A vectorized f32
epilogue computes the margin/softmax math for all tiles at once on
[128, n_tiles, 3] (sim_p from a separately-DMA'd exact f32 diagonal),
and per-(partition, tile) partial sums are DMA'd out as [128, n_tiles].
The final mean over the 8 * 128 * n_tiles partials is computed on host.

Budget (per core): DMA 16 MiB ~47us, Act ~6.4us/tile, DVE ~6.6us/tile,
PE ~5us/tile.
"""

import numpy as np
import ml_dtypes

import concourse.bacc as bacc
import concourse.mybir as mybir
import concourse.tile as tile
from concourse.bass_utils import run_bass_kernel_spmd

_B = 8192          # full problem size (rows == cols)
_NCORES = 8
_P = 128           # SBUF partitions
_K = 3
_MASK_W = -240.0   # e4m3 max; masked = x - 240*t sits below any real sim
_NEG_FILL = -50.0  # reference's softmax mask fill (must match exactly)
_INV_TAU = 10.0    # 1 / 0.1
_FOLD_W = 512      # fold the row down to this width before Max8
_MM_FD = 512       # matmul chunk free dim (one DoubleRow pass)
_QCOLS = 2048      # PSUM quarter (4 banks); 2 quarters in flight
# Eviction split inside the last quarter of each tile: the Scalar engine
# handles everything up to this column, the DVE the rest, so both land
# near the DMA rate (~5.9us/tile).
_DVE_EVICT = 1024


def _fold_max(nc, m, width, out8):
    """Fold max by halves (bf16 2x mode) to _FOLD_W, Max8 into out8."""
    w = width
    while w > _FOLD_W:
        h = w // 2
        nc.vector.tensor_tensor(out=m[:, :h], in0=m[:, :h], in1=m[:, h:w],
                                op=mybir.AluOpType.max)
        w = h
    nc.vector.max(out=out8, in_=m[:, :w])


def _build_nc(rows_per_core: int, ncols: int) -> bacc.Bacc:
    n_tiles = rows_per_core // _P
    n_q = ncols // _QCOLS
    f32 = mybir.dt.float32
    bf16 = mybir.dt.bfloat16
    fp8 = mybir.dt.float8e4
    i32 = mybir.dt.int32

    nc = bacc.Bacc()
    # xt[r, 0, :] = input row r as fp8; xt[r, 1, :] = target row r as fp8
    xt = nc.dram_tensor("xt", [rows_per_core, 2, ncols], fp8,
                        kind="ExternalInput")
    # wgt[:, 0, :] = Id, wgt[:, 1, :] = -240*Id  (DoubleRow stationary)
    wgt = nc.dram_tensor("wgt", [_P, 2, _P], fp8, kind="ExternalInput")
    # diag[p, t] = input diagonal element of local row t*128 + p
    diag = nc.dram_tensor("diag", [_P, n_tiles], f32, kind="ExternalInput")
    out = nc.dram_tensor("out", [_P, n_tiles], f32, kind="ExternalOutput")

    with tile.TileContext(nc) as tc:
        with (
            tc.tile_pool(name="singles", bufs=1) as singles,
            tc.tile_pool(name="io_xt", bufs=3) as io_xt,
            tc.tile_pool(name="mbuf", bufs=2) as mpool,
            tc.psum_pool(name="pp", bufs=2) as pp,
            tc.tile_pool(name="small", bufs=1) as small,
        ):
            wsb = singles.tile([_P, 2, _P], fp8)
            nc.sync.dma_start(out=wsb, in_=wgt[:, :, :])
            # top-8 per (row, tile), filled by the main loop
            vfin = singles.tile([_P, n_tiles, 8], bf16)

            for t in range(n_tiles):
                rows = slice(t * _P, (t + 1) * _P)
                xt_t = io_xt.tile([_P, 2, ncols], fp8)
                m_t = mpool.tile([_P, ncols], bf16, tag="m")
                if t == 0:
                    # per-quarter DMA so PE starts on the first 2048 cols
                    for q in range(n_q):
                        qs = slice(q * _QCOLS, (q + 1) * _QCOLS)
                        nc.sync.dma_start(out=xt_t[:, :, qs],
                                          in_=xt[rows, :, qs])
                else:
                    nc.sync.dma_start(out=xt_t, in_=xt[rows, :, :])
                for q in range(n_q):
                    pt = pp.tile([_P, _QCOLS], f32)
                    for c in range(_QCOLS // _MM_FD):
                        col = q * _QCOLS + c * _MM_FD
                        nc.tensor.matmul(
                            out=pt[:, c * _MM_FD:(c + 1) * _MM_FD],
                            lhsT=wsb[:, :, :],
                            rhs=xt_t[:, :, col:col + _MM_FD],
                            start=True, stop=True,
                            perf_mode=mybir.MatmulPerfMode.DoubleRow)
                    q0 = q * _QCOLS
                    if q < n_q - 1:
                        nc.scalar.copy(out=m_t[:, q0:q0 + _QCOLS], in_=pt)
                    else:
                        # split the last quarter's eviction Act/DVE
                        nc.scalar.copy(out=m_t[:, q0:q0 + _DVE_EVICT],
                                       in_=pt[:, :_DVE_EVICT])
                        nc.vector.tensor_copy(
                            out=m_t[:, q0 + _DVE_EVICT:q0 + _QCOLS],
                            in_=pt[:, _DVE_EVICT:])
                _fold_max(nc, m_t, ncols, vfin[:, t, :])

            # ---- vectorized epilogue over all tiles: [128, n_tiles, 3] ----
            diag_raw = singles.tile([_P, n_tiles], f32)
            nc.sync.dma_start(out=diag_raw, in_=diag[:, :])
            diag_sb = singles.tile([_P, n_tiles], f32)
            nc.vector.tensor_copy(out=diag_sb, in_=diag_raw)
            sh = [_P, n_tiles, _K]
            v = small.tile(sh, f32)                    # top-3, descending
            nc.vector.tensor_copy(out=v, in_=vfin[:, :, 0:_K])
            p_b = diag_sb.unsqueeze(-1).to_broadcast(sh)

            x = small.tile(sh, f32)                    # x = sim_n - sim_p
            nc.vector.tensor_tensor(out=x, in0=v, in1=p_b,
                                    op=mybir.AluOpType.subtract)
            # a = clip(|x|, 0.1, 0.3)   (|x| as max(x, -x), bitwise exact)
            negx = small.tile(sh, f32)
            nc.vector.tensor_scalar(out=negx, in0=x, scalar1=-1.0,
                                    scalar2=None, op0=mybir.AluOpType.mult)
            a = small.tile(sh, f32)
            nc.vector.tensor_tensor(out=a, in0=x, in1=negx,
                                    op=mybir.AluOpType.max)
            nc.vector.tensor_scalar(out=a, in0=a, scalar1=0.1, scalar2=0.3,
                                    op0=mybir.AluOpType.max,
                                    op1=mybir.AluOpType.min)
            # loss = relu(x + a); active = (x + a) > 0
            xa = small.tile(sh, f32)
            nc.vector.tensor_tensor(out=xa, in0=x, in1=a,
                                    op=mybir.AluOpType.add)
            l = small.tile(sh, f32)
            nc.vector.tensor_scalar(out=l, in0=xa, scalar1=0.0, scalar2=None,
                                    op0=mybir.AluOpType.max)
            act = small.tile(sh, i32)
            nc.vector.tensor_scalar(out=act, in0=xa, scalar1=0.0, scalar2=None,
                                    op0=mybir.AluOpType.is_gt)
            # s = where(active, v, -50)
            s = small.tile(sh, f32)
            nc.vector.memset(s, _NEG_FILL)
            nc.vector.copy_predicated(out=s, mask=act, data=v)
            # softmax(s / tau) over K, with max-subtraction (matches jax)
            smax = small.tile([_P, n_tiles], f32)
            nc.vector.reduce_max(out=smax, in_=s, axis=mybir.AxisListType.X)
            s2 = small.tile(sh, f32)
            nc.vector.tensor_tensor(out=s2, in0=s,
                                    in1=smax.unsqueeze(-1).to_broadcast(sh),
                                    op=mybir.AluOpType.subtract)
            e = small.tile(sh, f32)
            nc.scalar.activation(out=e, in_=s2,
                                 func=mybir.ActivationFunctionType.Exp,
                                 scale=_INV_TAU)
            z = small.tile([_P, n_tiles], f32)
            nc.vector.reduce_sum(out=z, in_=e, axis=mybir.AxisListType.X)
            r = small.tile([_P, n_tiles], f32)
            nc.vector.reciprocal(out=r, in_=z)
            w = small.tile(sh, f32)
            nc.vector.tensor_tensor(out=w, in0=e,
                                    in1=r.unsqueeze(-1).to_broadcast(sh),
                                    op=mybir.AluOpType.mult)
            lw = small.tile(sh, f32)
            nc.vector.tensor_tensor(out=lw, in0=l, in1=w,
                                    op=mybir.AluOpType.mult)
            out_sb = small.tile([_P, n_tiles], f32)
            nc.vector.reduce_sum(out=out_sb, in_=lw, axis=mybir.AxisListType.X)
            nc.sync.dma_start(out=out[:, :], in_=out_sb)
    nc.compile()
    return nc


def _prepare_in_maps(inp: np.ndarray, tgt: np.ndarray, ncores: int):
    b, ncols = inp.shape
    rows = b // ncores
    n_tiles = rows // _P
    fp8 = ml_dtypes.float8_e4m3
    d = np.ascontiguousarray(np.diagonal(inp)).astype(np.float32, copy=False)
    # 0/1 int32 little-endian: byte 0 of each element carries the value
    tgt_v = tgt.view(np.int8)[:, ::4]
    # DoubleRow stationary operand: [Id; -240*Id], both columns per cell
    wgt = np.zeros((_P, 2, _P), dtype=fp8)
    idx = np.arange(_P)
    wgt[idx, 0, idx] = fp8(1.0)
    wgt[idx, 1, idx] = fp8(_MASK_W)
    in_maps = []
    for c in range(ncores):
        sl = slice(c * rows, (c + 1) * rows)
        xt = np.empty((rows, 2, ncols), dtype=fp8)
        xt[:, 0, :] = inp[sl].astype(fp8)
        xt[:, 1, :] = tgt_v[sl].astype(fp8)
        diag_c = np.ascontiguousarray(d[sl].reshape(n_tiles, _P).T)
        in_maps.append({
            "xt": xt,
            "wgt": wgt,
            "diag": diag_c,
        })
    return in_maps


_NC_CACHE = {}


def kernel(input, target):
    inp = np.asarray(input, dtype=np.float32)
    tgt = np.asarray(target, dtype=np.int32)
    b, ncols = inp.shape

    key = (b, ncols)
    nc = _NC_CACHE.get(key)
    if nc is None:
        nc = _NC_CACHE[key] = _build_nc(b // _NCORES, ncols)
    in_maps = _prepare_in_maps(inp, tgt, _NCORES)
    res = run_bass_kernel_spmd(nc, in_maps, list(range(_NCORES)))
    total = 0.0
    for r in res.results:
        total += r["out"].astype(np.float64).sum()
    return np.asarray(total / (b * _K), dtype=np.float32)


if __name__ == "__main__":
    rng = np.random.default_rng(0)
    b = _B
    x = rng.standard_normal((b, b), dtype=np.float32)
    t = rng.integers(0, 2, size=(b, b)).astype(np.int32)
    np.fill_diagonal(t, 1)
    print(kernel(x, t))


# revision 12
# speedup vs baseline: 1.7155x; 1.0211x over previous
"""Trainium2 Bass kernel for the ATripletMarginLossOHNMDM loss.

Per row i of an (B, B) input:
  sim_p      = input[i, i]
  masked     = where(target[i]==0, input[i], -big)
  sim_n[0:3] = top-3 values of masked          (hard negatives)
  d          = clip(|sim_p - sim_n|, 0.1, 0.3)
  loss       = relu(sim_n - sim_p + d)
  s          = where(loss>0, sim_n, -50)
  w          = softmax(s / 0.1)      (with max-subtraction, as jax.nn.softmax)
  out        = mean over (B, 3) of loss * w

Sharded by rows across 8 NeuronCores (1024 rows each). The rel-err
budget (2e-2) dwarfs fp8-e4m3 rounding of the candidate values
(measured 1.0e-3 end to end on the real inputs), which unlocks the
memory-bound optimum: ship the input and the 0/1 target as fp8,
16 MiB/core instead of the f32+int8 40 MiB — the HBM-per-core limit
(~358 GB/s) makes DMA the roofline at ~47us. x8 and t8 are separate
contiguous DRAM tensors (strided combined layouts measured ~9% below
line rate) DMA'd into one [128, 2, ncols] SBUF tile.

The mask runs on the otherwise-idle TensorEngine as ONE DoubleRow fp8
matmul per 512-column chunk: lhsT = [Id; -240*Id] (fp8, [128,2,128]),
rhs = [x8; t8] ([128,2,512] slices), so PSUM accumulates
masked = x - 240*t in f32 exactly (Id picks x through the array; -240
is the e4m3 max, far below any N(0,1) similarity). ~380ns/chunk,
~6.1us/tile.

The PSUM->SBUF eviction (1 elem/cycle/lane on any engine) is the
structural wall; it is split so every engine sits near the DMA rate:
  - Scalar engine: activation-Copy quarters 0-2 to bf16 (~5.9us/tile)
  - DVE: quarter 3 is evicted FUSED with its fold-1 contribution — one
    1x tensor_tensor max(PSUM q3, SBUF q1) (2048 cyc) — then fold2+
    in bf16 2x packed mode down to 512 columns and one Max8 (~5.6us)
  - GPSIMD: the other fold-1 half, max(q0, q2) in SBUF bf16 (~4.3us)
Max8 never gets a DVE fast mode, so feeding it 512 instead of 8192
columns is 16x cheaper; fold slot j aggregates columns {j, j+512, ...}
and a true top-3 element is missed only when two of them collide in one
slot (~0.5% of rows, miss substitutes the 4th-largest — sub-1e-4 effect
on the final mean).

Tile 0 is processed per 2048-column quarter (DMA -> 4 matmuls -> evict)
so PE work starts as soon as the first quarter lands. A vectorized f32
epilogue computes the margin/softmax math for all tiles at once on
[128, n_tiles, 3] (sim_p from a separately-DMA'd exact f32 diagonal),
and per-(partition, tile) partial sums are DMA'd out as [128, n_tiles].
The final mean over the 8 * 128 * n_tiles partials is computed on host.
"""

import numpy as np
import ml_dtypes

import concourse.bacc as bacc
import concourse.mybir as mybir
import concourse.tile as tile
from concourse.bass_utils import run_bass_kernel_spmd

_B = 8192          # full problem size (rows == cols)
_NCORES = 8
_P = 128           # SBUF partitions
_K = 3
_MASK_W = -240.0   # e4m3 max; masked = x - 240*t sits below any real sim
_NEG_FILL = -50.0  # reference's softmax mask fill (must match exactly)
_INV_TAU = 10.0    # 1 / 0.1
_FOLD_W = 512      # fold the row down to this width before Max8
_MM_FD = 512       # matmul chunk free dim (one DoubleRow pass)
_QCOLS = 2048      # PSUM quarter (4 banks); 2 quarters in flight


def _build_nc(rows_per_core: int, ncols: int) -> bacc.Bacc:
    n_tiles = rows_per_core // _P
    n_q = ncols // _QCOLS
    f32 = mybir.dt.float32
    bf16 = mybir.dt.bfloat16
    fp8 = mybir.dt.float8e4
    i32 = mybir.dt.int32

    nc = bacc.Bacc()
    x8 = nc.dram_tensor("x8", [rows_per_core, ncols], fp8,
                        kind="ExternalInput")
    t8 = nc.dram_tensor("t8", [rows_per_core, ncols], fp8,
                        kind="ExternalInput")
    # wgt[:, 0, :] = Id, wgt[:, 1, :] = -240*Id  (DoubleRow stationary)
    wgt = nc.dram_tensor("wgt", [_P, 2, _P], fp8, kind="ExternalInput")
    # diag[p, t] = input diagonal element of local row t*128 + p
    diag = nc.dram_tensor("diag", [_P, n_tiles], f32, kind="ExternalInput")
    out = nc.dram_tensor("out", [_P, n_tiles], f32, kind="ExternalOutput")

    with tile.TileContext(nc) as tc:
        with (
            tc.tile_pool(name="singles", bufs=1) as singles,
            tc.tile_pool(name="io_xt", bufs=3) as io_xt,
            tc.tile_pool(name="mbuf", bufs=2) as mpool,
            tc.psum_pool(name="pp", bufs=2) as pp,
            tc.tile_pool(name="small", bufs=1) as small,
        ):
            wsb = singles.tile([_P, 2, _P], fp8)
            nc.sync.dma_start(out=wsb, in_=wgt[:, :, :])
            # top-8 per (row, tile), filled by the main loop
            vfin = singles.tile([_P, n_tiles, 8], bf16)

            for t in range(n_tiles):
                rows = slice(t * _P, (t + 1) * _P)
                xt_t = io_xt.tile([_P, 2, ncols], fp8)
                m_t = mpool.tile([_P, ncols], bf16, tag="m")
                if t == 0:
                    # per-quarter DMA so PE starts on the first 2048 cols
                    for q in range(n_q):
                        qs = slice(q * _QCOLS, (q + 1) * _QCOLS)
                        nc.sync.dma_start(out=xt_t[:, 0, qs],
                                          in_=x8[rows, qs])
                        nc.sync.dma_start(out=xt_t[:, 1, qs],
                                          in_=t8[rows, qs])
                else:
                    nc.sync.dma_start(out=xt_t[:, 0, :], in_=x8[rows, :])
                    nc.sync.dma_start(out=xt_t[:, 1, :], in_=t8[rows, :])
                pt_last = None
                for q in range(n_q):
                    pt = pp.tile([_P, _QCOLS], f32)
                    for c in range(_QCOLS // _MM_FD):
                        col = q * _QCOLS + c * _MM_FD
                        nc.tensor.matmul(
                            out=pt[:, c * _MM_FD:(c + 1) * _MM_FD],
                            lhsT=wsb[:, :, :],
                            rhs=xt_t[:, :, col:col + _MM_FD],
                            start=True, stop=True,
                            perf_mode=mybir.MatmulPerfMode.DoubleRow)
                    if q < n_q - 1:
                        q0 = q * _QCOLS
                        nc.scalar.copy(out=m_t[:, q0:q0 + _QCOLS], in_=pt)
                    else:
                        pt_last = pt
                # DVE: evict q3 fused with its fold1 half:
                #   m[q1] = max(psum q3, m[q1])   (cols j vs j+4096)
                nc.vector.tensor_tensor(
                    out=m_t[:, _QCOLS:2 * _QCOLS], in0=pt_last,
                    in1=m_t[:, _QCOLS:2 * _QCOLS], op=mybir.AluOpType.max)
                # the other fold1 half: m[q0] = max(m[q0], m[q2])
                # (bf16 2x; GPSIMD cannot help — Pool tensor_tensor has no
                # max opcode, only power / integer add/mult/sub)
                nc.vector.tensor_tensor(
                    out=m_t[:, 0:_QCOLS], in0=m_t[:, 0:_QCOLS],
                    in1=m_t[:, 2 * _QCOLS:3 * _QCOLS],
                    op=mybir.AluOpType.max)
                # DVE: fold 4096 -> _FOLD_W in bf16 2x mode, then Max8
                w = 2 * _QCOLS
                while w > _FOLD_W:
                    h = w // 2
                    nc.vector.tensor_tensor(
                        out=m_t[:, :h], in0=m_t[:, :h], in1=m_t[:, h:w],
                        op=mybir.AluOpType.max)
                    w = h
                nc.vector.max(out=vfin[:, t, :], in_=m_t[:, :w])

            # ---- vectorized epilogue over all tiles: [128, n_tiles, 3] ----
            diag_raw = singles.tile([_P, n_tiles], f32)
            nc.sync.dma_start(out=diag_raw, in_=diag[:, :])
            diag_sb = singles.tile([_P, n_tiles], f32)
            nc.vector.tensor_copy(out=diag_sb, in_=diag_raw)
            sh = [_P, n_tiles, _K]
            v = small.tile(sh, f32)                    # top-3, descending
            nc.vector.tensor_copy(out=v, in_=vfin[:, :, 0:_K])
            p_b = diag_sb.unsqueeze(-1).to_broadcast(sh)

            x = small.tile(sh, f32)                    # x = sim_n - sim_p
            nc.vector.tensor_tensor(out=x, in0=v, in1=p_b,
                                    op=mybir.AluOpType.subtract)
            # a = clip(|x|, 0.1, 0.3)  (Abs on the Scalar engine, off DVE)
            a = small.tile(sh, f32)
            nc.scalar.activation(out=a, in_=x,
                                 func=mybir.ActivationFunctionType.Abs)
            nc.vector.tensor_scalar(out=a, in0=a, scalar1=0.1, scalar2=0.3,
                                    op0=mybir.AluOpType.max,
                                    op1=mybir.AluOpType.min)
            # loss = relu(x + a); active = (x + a) > 0
            xa = small.tile(sh, f32)
            nc.vector.tensor_tensor(out=xa, in0=x, in1=a,
                                    op=mybir.AluOpType.add)
            l = small.tile(sh, f32)
            nc.vector.tensor_scalar(out=l, in0=xa, scalar1=0.0, scalar2=None,
                                    op0=mybir.AluOpType.max)
            act = small.tile(sh, i32)
            nc.vector.tensor_scalar(out=act, in0=xa, scalar1=0.0, scalar2=None,
                                    op0=mybir.AluOpType.is_gt)
            # s = where(active, v, -50)
            s = small.tile(sh, f32)
            nc.vector.memset(s, _NEG_FILL)
            nc.vector.copy_predicated(out=s, mask=act, data=v)
            # softmax(s / tau) over K, with max-subtraction (matches jax)
            smax = small.tile([_P, n_tiles], f32)
            nc.vector.reduce_max(out=smax, in_=s, axis=mybir.AxisListType.X)
            s2 = small.tile(sh, f32)
            nc.vector.tensor_tensor(out=s2, in0=s,
                                    in1=smax.unsqueeze(-1).to_broadcast(sh),
                                    op=mybir.AluOpType.subtract)
            e = small.tile(sh, f32)
            nc.scalar.activation(out=e, in_=s2,
                                 func=mybir.ActivationFunctionType.Exp,
                                 scale=_INV_TAU)
            z = small.tile([_P, n_tiles], f32)
            nc.vector.reduce_sum(out=z, in_=e, axis=mybir.AxisListType.X)
            r = small.tile([_P, n_tiles], f32)
            nc.vector.reciprocal(out=r, in_=z)
            w = small.tile(sh, f32)
            nc.vector.tensor_tensor(out=w, in0=e,
                                    in1=r.unsqueeze(-1).to_broadcast(sh),
                                    op=mybir.AluOpType.mult)
            lw = small.tile(sh, f32)
            nc.vector.tensor_tensor(out=lw, in0=l, in1=w,
                                    op=mybir.AluOpType.mult)
            out_sb = small.tile([_P, n_tiles], f32)
            nc.vector.reduce_sum(out=out_sb, in_=lw, axis=mybir.AxisListType.X)
            nc.sync.dma_start(out=out[:, :], in_=out_sb)
    nc.compile()
    return nc


def _prepare_in_maps(inp: np.ndarray, tgt: np.ndarray, ncores: int):
    b, ncols = inp.shape
    rows = b // ncores
    n_tiles = rows // _P
    fp8 = ml_dtypes.float8_e4m3
    d = np.ascontiguousarray(np.diagonal(inp)).astype(np.float32, copy=False)
    # 0/1 int32 little-endian: byte 0 of each element carries the value
    tgt_v = tgt.view(np.int8)[:, ::4]
    # DoubleRow stationary operand: [Id; -240*Id], both columns per cell
    wgt = np.zeros((_P, 2, _P), dtype=fp8)
    idx = np.arange(_P)
    wgt[idx, 0, idx] = fp8(1.0)
    wgt[idx, 1, idx] = fp8(_MASK_W)
    in_maps = []
    for c in range(ncores):
        sl = slice(c * rows, (c + 1) * rows)
        diag_c = np.ascontiguousarray(d[sl].reshape(n_tiles, _P).T)
        in_maps.append({
            "x8": inp[sl].astype(fp8),
            "t8": tgt_v[sl].astype(fp8),
            "wgt": wgt,
            "diag": diag_c,
        })
    return in_maps


_NC_CACHE = {}


def kernel(input, target):
    inp = np.asarray(input, dtype=np.float32)
    tgt = np.asarray(target, dtype=np.int32)
    b, ncols = inp.shape

    key = (b, ncols)
    nc = _NC_CACHE.get(key)
    if nc is None:
        nc = _NC_CACHE[key] = _build_nc(b // _NCORES, ncols)
    in_maps = _prepare_in_maps(inp, tgt, _NCORES)
    res = run_bass_kernel_spmd(nc, in_maps, list(range(_NCORES)))
    total = 0.0
    for r in res.results:
        total += r["out"].astype(np.float64).sum()
    return np.asarray(total / (b * _K), dtype=np.float32)


if __name__ == "__main__":
    rng = np.random.default_rng(0)
    b = _B
    x = rng.standard_normal((b, b), dtype=np.float32)
    t = rng.integers(0, 2, size=(b, b)).astype(np.int32)
    np.fill_diagonal(t, 1)
    print(kernel(x, t))
